# revision 22
# baseline (speedup 1.0000x reference)
"""GAT edge->relation aggregation (nn_GAT_E_to_R) on 8 trn2 NeuronCores.

Strategy (edge sharding by relation):
  - Host: sort edges by relation id; assign each core a contiguous range of
    relations (~125 rels, ~200k edges). All softmax groups are then wholly
    local to one core: no collectives, output rows are disjoint.
  - Device phase A (replicated): project x_e with augmented weights
    [W_h|Wh@a_h1|Wh@a_t1 ; W_t|Wt@a_h2|Wt@a_t2] into two node tables of
    512-byte rows [x_r(128), s1, s2, 1.0, pad] (bf16, 256 cols) in DRAM.
  - Device phase B: per batch of 32x128 edges, dma_gather the h-row and
    t-row per edge (indices are int16, so tables are addressed with a
    parity trick: row n at stride-1024B offset (n>>1) from base (n&1)*512B;
    host pre-buckets each core's edges into 4 groups by (h&1, t&1) so each
    gather call has compile-time base/stride). Per-edge ex =
    exp(leaky_relu(s_a+s_b)) on DVE/ACT, ex-scaled one-hot(rel) built in
    bulk on DVE, then PE matmuls accumulate sum_e ex_e * [row_e..., 1.0]
    into PSUM windows [128 rel-slots, 131]; the 1.0 table column folds the
    softmax denominator into the same matmul.
  - Phase C: out[slot] = num1/z1 + num2/z2; host concatenates 8 slices.
"""

import numpy as np

N_NODES = 50000
E_EDGES = 1600000
E_HIDDEN = 256
R_HIDDEN = 128
R_RELS = 1000
NCORES = 8

P = 128
TAB_COLS = 130  # matmul output columns per table (128 feats + 2 scores)
TAB_ST = 256  # stored table row length (512 B): feats, s1, s2, 1.0, pad
NB = 32  # edge tiles (of 128 edges) per gather batch
NODE_BLK = 512  # node block for transposed x_e loads

_prog_cache = {}
LAST_RESULTS = None


def _build_program(n_pad, group_nts, e_hidden=E_HIDDEN, debug_dump=False, parity=True):
    """Build the single SPMD program (identical on all cores).

    group_nts: tiles (of 128 edges) per (h&1, t&1) parity group; each a
    multiple of NB. Same on every core (host pads to the max).
    """
    import concourse.bacc as bacc
    import concourse.tile as tile
    from concourse import bass, mybir

    dt = mybir.dt
    nt = sum(group_nts)
    assert all(g % NB == 0 for g in group_nts)
    assert n_pad % P == 0 and n_pad % 2 == 0
    kslab = e_hidden // P  # 2
    nbatch_total = nt // NB

    nc = bacc.Bacc(None, target_bir_lowering=False, debug=False)

    xe = nc.declare_dram_parameter("xe", [n_pad, e_hidden], dt.bfloat16, isOutput=False)
    w = nc.declare_dram_parameter("w", [e_hidden, 2 * TAB_COLS], dt.bfloat16, isOutput=False)
    # int16 gather indices, 16-wrapped and 8x replicated: per batch of
    # NB*128 idxs a [128, NB*8] block; concatenated along free dim.
    hidx = nc.declare_dram_parameter("hidx", [P, nt * 8], dt.int16, isOutput=False)
    tidx = nc.declare_dram_parameter("tidx", [P, nt * 8], dt.int16, isOutput=False)
    relv = nc.declare_dram_parameter("relv", [P, nt], dt.bfloat16, isOutput=False)
    iota = nc.declare_dram_parameter("iota", [P, P], dt.bfloat16, isOutput=False)
    out = nc.declare_dram_parameter("out", [P, R_HIDDEN], dt.float32, isOutput=True)
    if debug_dump:
        dbg_tabh = nc.declare_dram_parameter(
            "dbg_tabh", [n_pad, TAB_ST], dt.bfloat16, isOutput=True
        )
        dbg_gh = nc.declare_dram_parameter(
            "dbg_gh", [P, NB * TAB_ST], dt.bfloat16, isOutput=True
        )
        dbg_ex1 = nc.declare_dram_parameter(
            "dbg_ex1", [P, NB], dt.bfloat16, isOutput=True
        )
        dbg_oh1 = nc.declare_dram_parameter(
            "dbg_oh1", [P, NB * P], dt.bfloat16, isOutput=True
        )
        dbg_ps1 = nc.declare_dram_parameter(
            "dbg_ps1", [P, TAB_COLS + 1], dt.float32, isOutput=True
        )

    tabh = nc.dram_tensor("tabh", [n_pad, TAB_ST], dt.bfloat16)
    tabt = nc.dram_tensor("tabt", [n_pad, TAB_ST], dt.bfloat16)

    AL = mybir.AluOpType
    AF = mybir.ActivationFunctionType

    with tile.TileContext(nc) as tc:
        with (
            tc.tile_pool(name="const", bufs=1) as constp,
            tc.tile_pool(name="psout", bufs=1, space="PSUM") as psoutp,
        ):
            w_t = constp.tile([P, kslab, 2 * TAB_COLS], dt.bfloat16)
            nc.sync.dma_start(out=w_t[:], in_=w[:].rearrange("(s p) c -> p s c", p=P))
            iota_t = constp.tile([P, P], dt.bfloat16)
            nc.sync.dma_start(out=iota_t[:], in_=iota[:])

            # ---------------- Phase A: node tables ----------------
            with (
                tc.tile_pool(name="xt", bufs=3) as xtp,
                tc.tile_pool(name="stage", bufs=4) as stp,
                tc.tile_pool(name="pproj", bufs=4, space="PSUM") as pprojp,
            ):
                for n0 in range(0, n_pad, NODE_BLK):
                    nn = min(NODE_BLK, n_pad - n0)
                    xts = []
                    for s in range(kslab):
                        xt_s = xtp.tile([P, NODE_BLK], dt.bfloat16, tag=f"xt{s}")
                        nc.sync.dma_start(
                            out=xt_s[:, :nn],
                            in_=xe[n0 : n0 + nn, s * P : (s + 1) * P],
                            transpose=True,
                        )
                        xts.append(xt_s)
                    for sub in range(nn // P):
                        ps = pprojp.tile([P, 2 * TAB_COLS], dt.float32, tag="pj")
                        for s in range(kslab):
                            nc.tensor.matmul(
                                out=ps[:],
                                lhsT=xts[s][:, sub * P : (sub + 1) * P],
                                rhs=w_t[:, s, :],
                                start=(s == 0),
                                stop=(s == kslab - 1),
                            )
                        sg = stp.tile([P, 2, TAB_ST], dt.bfloat16, tag="sg")
                        nc.vector.tensor_copy(
                            out=sg[:, :, 0:TAB_COLS],
                            in_=ps[:].rearrange("p (s c) -> p s c", s=2),
                        )
                        nc.vector.memset(sg[:, :, TAB_COLS], 1.0)
                        nc.gpsimd.memset(sg[:, :, TAB_COLS + 1 :], 0.0)
                        c0 = n0 + sub * P
                        nc.sync.dma_start(out=tabh[c0 : c0 + P, :], in_=sg[:, 0, :])
                        nc.sync.dma_start(out=tabt[c0 : c0 + P, :], in_=sg[:, 1, :])

            tc.strict_bb_all_engine_barrier()
            if debug_dump:
                nc.sync.dma_start(out=dbg_tabh[:], in_=tabh[:])

            # parity views of the tables: row n -> view[p][n>>1] where
            # p = n&1. [n_pad//2, TAB_ST] at row stride 2*TAB_ST.
            if parity:
                tabh_r = tabh[:].rearrange("(n two) c -> n (two c)", two=2)
                tabt_r = tabt[:].rearrange("(n two) c -> n (two c)", two=2)
                tab_par = {
                    ("h", 0): tabh_r[:, 0:TAB_ST],
                    ("h", 1): tabh_r[:, TAB_ST : 2 * TAB_ST],
                    ("t", 0): tabt_r[:, 0:TAB_ST],
                    ("t", 1): tabt_r[:, TAB_ST : 2 * TAB_ST],
                }
                gstep = 2 * TAB_ST
            else:
                tab_par = {
                    ("h", 0): tabh[:],
                    ("h", 1): tabh[:],
                    ("t", 0): tabt[:],
                    ("t", 1): tabt[:],
                }
                gstep = TAB_ST

            # ---------------- Phase B: edge aggregation ----------------
            ps1 = psoutp.tile([P, TAB_COLS + 1], dt.float32, tag="ps1")
            ps2 = psoutp.tile([P, TAB_COLS + 1], dt.float32, tag="ps2")
            with (
                tc.tile_pool(name="idx", bufs=3) as idxp,
                tc.tile_pool(name="gath", bufs=2) as gp,
                tc.tile_pool(name="ohe", bufs=1) as ohep,
                tc.tile_pool(name="ohs", bufs=2) as ohsp,
                tc.tile_pool(name="sc", bufs=3) as scp,
            ):
                NIC = NB * 8  # idx cols per batch in the wrapped layout
                bglobal = 0
                for g in range(4):
                    hp, tp = g & 1, (g >> 1) & 1
                    for _bg in range(group_nts[g] // NB):
                        b = bglobal
                        bglobal += 1
                        first_b = b == 0
                        last_b = b == nbatch_total - 1
                        ht = idxp.tile([P, NIC], dt.int16, tag="ht")
                        tt = idxp.tile([P, NIC], dt.int16, tag="tt")
                        rt = idxp.tile([P, NB], dt.bfloat16, tag="rt")
                        nc.sync.dma_start(
                            out=ht[:], in_=hidx[:, b * NIC : (b + 1) * NIC]
                        )
                        nc.sync.dma_start(
                            out=tt[:], in_=tidx[:, b * NIC : (b + 1) * NIC]
                        )
                        nc.sync.dma_start(
                            out=rt[:], in_=relv[:, b * NB : (b + 1) * NB]
                        )

                        gh = gp.tile([P, NB, TAB_ST], dt.bfloat16, tag="gh")
                        gt = gp.tile([P, NB, TAB_ST], dt.bfloat16, tag="gt")
                        nc.gpsimd.dma_gather(
                            out_ap=gh[:],
                            in_ap=tab_par[("h", hp)],
                            idxs_ap=ht[:],
                            num_idxs=NB * P,
                            num_idxs_reg=NB * P,
                            elem_size=TAB_ST,
                            elem_step=gstep,
                            single_packet=False,
                        )
                        nc.gpsimd.dma_gather(
                            out_ap=gt[:],
                            in_ap=tab_par[("t", tp)],
                            idxs_ap=tt[:],
                            num_idxs=NB * P,
                            num_idxs_reg=NB * P,
                            elem_size=TAB_ST,
                            elem_step=gstep,
                            single_packet=False,
                        )

                        # scores: e1 = s_h1[h]+s_h2[t]; e2 = s_t1[h]+s_t2[t]
                        ex = []
                        for k in range(2):
                            col = R_HIDDEN + k
                            e_k = scp.tile([P, NB], dt.float32, tag=f"e{k}")
                            nc.vector.tensor_tensor(
                                out=e_k[:],
                                in0=gh[:, :, col],
                                in1=gt[:, :, col],
                                op=AL.add,
                            )
                            l_k = scp.tile([P, NB], dt.float32, tag=f"l{k}")
                            nc.vector.scalar_tensor_tensor(
                                out=l_k[:],
                                in0=e_k[:],
                                scalar=0.01,
                                in1=e_k[:],
                                op0=AL.mult,
                                op1=AL.max,
                            )
                            x_k = scp.tile([P, NB], dt.bfloat16, tag=f"x{k}")
                            nc.scalar.activation(out=x_k[:], in_=l_k[:], func=AF.Exp)
                            ex.append(x_k)

                        ohe = ohep.tile([P, NB, P], dt.bfloat16, tag="ohe")
                        nc.vector.tensor_tensor(
                            out=ohe[:],
                            in0=iota_t[:, None, :].to_broadcast([P, NB, P]),
                            in1=rt[:, :, None].to_broadcast([P, NB, P]),
                            op=AL.is_equal,
                        )
                        oh1 = ohsp.tile([P, NB, P], dt.bfloat16, tag="oh1")
                        oh2 = ohsp.tile([P, NB, P], dt.bfloat16, tag="oh2")
                        nc.vector.tensor_tensor(
                            out=oh1[:],
                            in0=ohe[:],
                            in1=ex[0][:, :, None].to_broadcast([P, NB, P]),
                            op=AL.mult,
                        )
                        nc.vector.tensor_tensor(
                            out=oh2[:],
                            in0=ohe[:],
                            in1=ex[1][:, :, None].to_broadcast([P, NB, P]),
                            op=AL.mult,
                        )

                        if debug_dump and first_b:
                            nc.sync.dma_start(
                                out=dbg_gh[:], in_=gh[:].rearrange("p a c -> p (a c)")
                            )
                            nc.sync.dma_start(out=dbg_ex1[:], in_=ex[0][:])
                            nc.sync.dma_start(
                                out=dbg_oh1[:], in_=oh1[:].rearrange("p a c -> p (a c)")
                            )
                        for j in range(NB):
                            first = first_b and j == 0
                            last = last_b and j == NB - 1
                            nc.tensor.matmul(
                                out=ps1[:],
                                lhsT=oh1[:, j, :],
                                rhs=gh[:, j, 0 : TAB_COLS + 1],
                                start=first,
                                stop=last,
                            )
                            nc.tensor.matmul(
                                out=ps2[:],
                                lhsT=oh2[:, j, :],
                                rhs=gt[:, j, 0 : TAB_COLS + 1],
                                start=first,
                                stop=last,
                            )

                # ---------------- Phase C: divide + store ----------------
                fin_o = []
                for k, psk in enumerate((ps1, ps2)):
                    zb = scp.tile([P, 1], dt.float32, tag=f"zb{k}")
                    nc.vector.tensor_scalar(
                        out=zb[:],
                        in0=psk[:, TAB_COLS : TAB_COLS + 1],
                        scalar1=1e-6,
                        scalar2=None,
                        op0=AL.add,
                    )
                    zr = scp.tile([P, 1], dt.float32, tag=f"zr{k}")
                    nc.vector.reciprocal(out=zr[:], in_=zb[:])
                    o_k = scp.tile([P, R_HIDDEN], dt.float32, tag=f"o{k}")
                    nc.vector.tensor_scalar(
                        out=o_k[:],
                        in0=psk[:, 0:R_HIDDEN],
                        scalar1=zr[:],
                        scalar2=None,
                        op0=AL.mult,
                    )
                    fin_o.append(o_k)
                oo = scp.tile([P, R_HIDDEN], dt.float32, tag="oo")
                nc.vector.tensor_tensor(
                    out=oo[:], in0=fin_o[0][:], in1=fin_o[1][:], op=AL.add
                )
                nc.sync.dma_start(out=out[:], in_=oo[:])
                if debug_dump:
                    pd = scp.tile([P, TAB_COLS + 1], dt.float32, tag="pd")
                    nc.vector.tensor_copy(out=pd[:], in_=ps1[:])
                    nc.sync.dma_start(out=dbg_ps1[:], in_=pd[:])

    nc.compile()
    return nc


def _wrap_idx16(arr):
    """int16 gather indices -> [128, n*8/16... ] wrapped-16, replicated x8,
    per batch of NB*128."""
    nbatch = arr.shape[0] // (NB * P)
    cols = []
    for k in range(nbatch):
        blk = arr[k * NB * P : (k + 1) * NB * P]
        w16 = blk.reshape(-1, 16).T  # [16, NB*8]
        cols.append(np.tile(w16, (8, 1)))
    return np.ascontiguousarray(np.concatenate(cols, axis=1))


def _wrap_tile(arr):
    """per-edge values -> [128, ntiles]: edge j at [j%128, j//128]."""
    return np.ascontiguousarray(arr.reshape(-1, P).T)


def _prep_core_edges(h, t, rl, group_nts, parity=True):
    """Bucket one core's edges by (h&1, t&1), pad each group to
    group_nts[g]*128 edges, and build the device index/rel arrays."""
    g_id = (h & 1) + 2 * (t & 1)
    hs, ts, rs = [], [], []
    for g in range(4):
        sel = g_id == g
        hg, tg, rg = h[sel], t[sel], rl[sel]
        npad = group_nts[g] * P - hg.shape[0]
        assert npad >= 0
        hp, tp = g & 1, (g >> 1) & 1
        hs.append(np.concatenate([hg, np.full(npad, hp, h.dtype)]))
        ts.append(np.concatenate([tg, np.full(npad, tp, t.dtype)]))
        rs.append(np.concatenate([rg, np.full(npad, 127, rl.dtype)]))
    hc = np.concatenate(hs)
    tc = np.concatenate(ts)
    rc = np.concatenate(rs)
    sh = 1 if parity else 0
    return {
        "hidx": _wrap_idx16((hc >> sh).astype(np.int16)),
        "tidx": _wrap_idx16((tc >> sh).astype(np.int16)),
        "relv": _wrap_tile(rc),
    }


def _host_prep(x_e, edge_index, rel, W_h, W_t, a_h1, a_h2, a_t1, a_t2):
    import ml_dtypes

    bf16 = ml_dtypes.bfloat16
    E = rel.shape[0]
    n = x_e.shape[0]
    n_pad = ((n + P - 1) // P) * P

    h = np.asarray(edge_index[0], dtype=np.int64)
    t = np.asarray(edge_index[1], dtype=np.int64)
    r = np.asarray(rel, dtype=np.int64)

    order = np.argsort(r, kind="stable")
    hs = h[order].astype(np.int32)
    ts = t[order].astype(np.int32)
    rs = r[order].astype(np.int32)

    counts = np.bincount(rs, minlength=R_RELS)
    cum = np.concatenate([[0], np.cumsum(counts)])
    br = [0]
    for i in range(1, NCORES):
        k = int(np.argmin(np.abs(cum - E * i / NCORES)))
        br.append(k)
    br.append(R_RELS)
    nrels = [br[i + 1] - br[i] for i in range(NCORES)]
    assert all(0 < x <= 127 for x in nrels), nrels
    ebounds = [int(cum[k]) for k in br]

    # per-core parity-group sizes -> common padded group_nts
    core_groups = []
    gmax = [0, 0, 0, 0]
    for i in range(NCORES):
        lo, hi = ebounds[i], ebounds[i + 1]
        g_id = (hs[lo:hi] & 1) + 2 * (ts[lo:hi] & 1)
        sizes = np.bincount(g_id, minlength=4)
        core_groups.append(sizes)
        for g in range(4):
            gmax[g] = max(gmax[g], int(sizes[g]))
    group_nts = tuple(
        ((gm + NB * P - 1) // (NB * P)) * NB for gm in gmax
    )  # tiles per group

    w_aug = np.zeros((E_HIDDEN, 2 * TAB_COLS), dtype=np.float32)
    w_aug[:, 0:R_HIDDEN] = W_h
    w_aug[:, R_HIDDEN] = W_h @ a_h1
    w_aug[:, R_HIDDEN + 1] = W_h @ a_t1
    w_aug[:, TAB_COLS : TAB_COLS + R_HIDDEN] = W_t
    w_aug[:, TAB_COLS + R_HIDDEN] = W_t @ a_h2
    w_aug[:, TAB_COLS + R_HIDDEN + 1] = W_t @ a_t2

    xe_pad = np.zeros((n_pad, E_HIDDEN), dtype=np.float32)
    xe_pad[:n] = x_e
    xe_bf = xe_pad.astype(bf16)
    w_bf = w_aug.astype(bf16)
    iota_np = np.tile(np.arange(P, dtype=np.float32), (P, 1)).astype(bf16)

    in_maps = []
    for i in range(NCORES):
        lo, hi = ebounds[i], ebounds[i + 1]
        rl_local = (rs[lo:hi] - br[i]).astype(np.float32).astype(bf16)
        m = _prep_core_edges(hs[lo:hi], ts[lo:hi], rl_local, group_nts)
        m.update({"xe": xe_bf, "w": w_bf, "iota": iota_np})
        in_maps.append(m)
    return in_maps, br, nrels, group_nts, n_pad


def kernel(x_e, edge_index, rel, W_h, W_t, a_h1, a_h2, a_t1, a_t2, _trace=False):
    global LAST_RESULTS
    from concourse.bass_utils import run_bass_kernel_spmd

    in_maps, br, nrels, group_nts, n_pad = _host_prep(
        x_e, edge_index, rel, W_h, W_t, a_h1, a_h2, a_t1, a_t2
    )
    key = (n_pad, group_nts)
    if key not in _prog_cache:
        _prog_cache[key] = _build_program(n_pad, group_nts)
    nc = _prog_cache[key]

    res = run_bass_kernel_spmd(nc, in_maps, list(range(NCORES)), trace=_trace)
    LAST_RESULTS = res

    out = np.zeros((R_RELS, R_HIDDEN), dtype=np.float32)
    for i in range(NCORES):
        out[br[i] : br[i + 1]] = res.results[i]["out"][: nrels[i]]
    return out
